# revision 27
# baseline (speedup 1.0000x reference)
"""BitLinear (ternary weight + per-token int8 absmax activation) on 8 trn2 cores.

y = (round(x/s) clipped) * s  @  (alpha * clip(round(W/alpha),-1,1)).T  + bias
  with s = max(absmax(x, -1), eps)/127 per token, alpha = max(mean|W|, eps).

Strategy: data-parallel over tokens (4096 tokens/core).  W^T is precomputed
HOST-side (pure layout) and passed as an extra input, so no device transpose
of W is needed; each core ternarizes the full W^T locally (one 16.8 MB load).
alpha = mean|W| is computed from each core's 1/8 shard of the chunks it loads
anyway, combined with a 4-byte AllReduce (ALPHA_SHARD=True) or fully locally
from an extra 2.1 MB pass (False).

Matmul runs in fp8e4 DoubleRow perf mode with an EXACT split of the int8
activations:  q = a + b,  a = fp8e4(q) (RNE), b = q - a (|b| <= 4, exact in
fp8e4).  The DoubleRow dual-MAC computes w*a + w*b = w*q exactly; the ternary
weight is fed to both slots via a stride-0 broadcast AP (verified exact on
hw), so wT is stored un-duplicated (32 KB/partition) and double-buffered
across repeats.  PSUM accumulates in fp32: the integer matmul is bit-exact.

Rounding uses the magic-number trick (v + 1.5*2^23 - 1.5*2^23) == RNE.
y is scaled+biased in one fused DVE op (PSUM*c + bias) and stored as bf16
(host casts back to f32; |y|<=3 so bf16 keeps rel err ~2.6e-3 << 2e-2).
"""

import numpy as np
from contextlib import ExitStack

import concourse.bass as bass
from concourse import bacc
import concourse.mybir as mybir
import concourse.tile as tile
from concourse.bass import ts
from concourse.bass_utils import run_bass_kernel_spmd

P = 128
D_IN = 2048
D_OUT = 2048
KC = D_IN // P          # 16 contraction chunks
MAGIC = 12582912.0      # 1.5 * 2**23 : fp32 RNE rounding offset
EPS = 1e-5
CLAMP = float(np.nextafter(np.float32(1.5), np.float32(0.0)))  # largest f32 < 1.5
N_CORES = 8
ST = 2                  # token tiles per supertile

USE_FP8 = True          # fp8e4 DoubleRow exact-split matmul (else bf16)
NFREE = 512            # matmul moving free size (512 | 1024 | 2048)
Y_BF16 = True           # store y as bf16 (host upcasts)
ALPHA_SHARD = True      # alpha via 1/8 shard + tiny AllReduce (else local pass)

F32 = mybir.dt.float32
BF16 = mybir.dt.bfloat16
FP8 = mybir.dt.float8e4
Copy = mybir.ActivationFunctionType.Copy
Alu = mybir.AluOpType
AX = mybir.AxisListType
DR = mybir.MatmulPerfMode.DoubleRow
GROUPS = [list(range(N_CORES))]

NT = D_OUT // NFREE     # matmuls per (token tile, k)
YDT = BF16 if Y_BF16 else F32


def _build(T: int, repeat: int = 1) -> bass.Bass:
    """Build the per-core program for T tokens (repeat>1: perf timing only)."""
    st = ST if T % (P * ST) == 0 else 1
    MS = T // (P * st)  # supertiles
    nc = bacc.Bacc(None, target_bir_lowering=False)

    N_RED = 2 if ALPHA_SHARD else KC
    x_d = nc.dram_tensor("x", [T, D_IN], F32, kind="ExternalInput")
    wt_d = nc.dram_tensor("wt", [D_IN, D_OUT], F32, kind="ExternalInput")
    ws_d = nc.dram_tensor("ws", [N_RED * P, D_OUT], F32, kind="ExternalInput")
    b_d = nc.dram_tensor("b", [D_OUT], F32, kind="ExternalInput")
    y_d = nc.dram_tensor("y", [T, D_OUT], YDT, kind="ExternalOutput")
    x_v = x_d.rearrange("(s a p) d -> s p a d", p=P, a=st)
    y_v = y_d.rearrange("(s a p) d -> s p a d", p=P, a=st)

    with tile.TileContext(nc) as tc, ExitStack() as ctx:
      const = ctx.enter_context(tc.tile_pool(name="const", bufs=1))
      alph = ctx.enter_context(tc.tile_pool(name="alph", bufs=2))
      wpool = ctx.enter_context(tc.tile_pool(name="wpool", bufs=2))
      wload = ctx.enter_context(tc.tile_pool(name="wload", bufs=3))
      xin = ctx.enter_context(tc.tile_pool(name="xin", bufs=2))
      xq = ctx.enter_context(tc.tile_pool(name="xq", bufs=2))
      xt = ctx.enter_context(tc.tile_pool(name="xt", bufs=2))
      scl = ctx.enter_context(tc.tile_pool(name="scl", bufs=4))
      yout = ctx.enter_context(tc.tile_pool(name="yout", bufs=2))
      psum = ctx.enter_context(tc.tile_pool(name="psum", bufs=2, space="PSUM"))
      dram = ctx.enter_context(tc.tile_pool(name="dram", bufs=1, space="DRAM"))

      bias_bc = const.tile([P, D_OUT], F32)
      ones = const.tile([P, 1], F32)
      nc.gpsimd.dma_start(out=bias_bc[:], in_=b_d[None, :].to_broadcast((P, D_OUT)))
      nc.scalar.activation(ones[:], bias_bc[:, 0:1], Copy, scale=0.0, bias=1.0)

      for _rep in range(repeat):
        wT = wpool.tile([P, KC, D_OUT], FP8 if USE_FP8 else BF16, tag="wT")
        partial = alph.tile([P, N_RED], F32, tag="partial")
        alpha_sb = alph.tile([P, 1], F32, tag="alpha_sb")
        inv_alpha = alph.tile([P, 1], F32, tag="inv_alpha")
        alpha127 = alph.tile([P, 1], F32, tag="alpha127")

        # ---- phase W-A: alpha from this core's shard (ALPHA_SHARD) or the
        # full W (else); finished with a tiny AllReduce when sharded --------
        for c in range(N_RED):
            sc = wload.tile([P, D_OUT], F32, tag="wchunk")
            nc.sync.dma_start(out=sc[:], in_=ws_d[ts(c, P), :])
            s1 = scl.tile([P, KC], F32, tag="s1")
            nc.vector.tensor_reduce(
                s1[:], sc.rearrange("p (a b) -> p a b", a=KC), axis=AX.X,
                op=Alu.add, apply_absolute_value=True,
            )
            nc.vector.tensor_reduce(
                partial[:, c : c + 1], s1[:], axis=AX.X, op=Alu.add
            )
        # cross-partition sum via PE dot with ones (PE adder tree), then a
        # pairwise tree over the chunk sums.
        ps0 = psum.tile([P, NT, NFREE], F32, tag="ps")
        pdot = ps0.rearrange("p a b -> p (a b)")[0:1, 0:N_RED]
        nc.tensor.matmul(pdot, ones[:], partial[:], start=True, stop=True)
        row = alph.tile([1, KC], F32, tag="row")
        nc.scalar.copy(row[0:1, 0:N_RED], pdot)
        width = N_RED // 2
        while width >= 1:
            nc.vector.tensor_tensor(
                row[0:1, 0:width], row[0:1, 0:width],
                row[0:1, width : 2 * width], op=Alu.add,
            )
            width //= 2
        if ALPHA_SHARD:
            # tiny AllReduce of the per-core shard sum
            ar_in = dram.tile([1, 1], F32, name="ar_in")
            ar_out = dram.tile([1, 1], F32, name="ar_out", addr_space="Shared")
            nc.sync.dma_start(out=ar_in[:], in_=row[0:1, 0:1])
            nc.gpsimd.collective_compute(
                "AllReduce", Alu.add, replica_groups=GROUPS,
                ins=[ar_in[:]], outs=[ar_out[:]],
            )
            tot = alph.tile([1, 1], F32, tag="tot")
            nc.sync.dma_start(out=tot[:], in_=ar_out[:])
            src = tot
        else:
            src = row
        al_sc = alph.tile([1, 1], F32, tag="al_sc")
        nc.vector.tensor_scalar(
            al_sc[:], src[0:1, 0:1], 1.0 / (D_IN * D_OUT), EPS,
            op0=Alu.mult, op1=Alu.max,
        )
        # broadcast alpha to all partitions through a DRAM bounce
        al_d = dram.tile([1, 1], F32, name="al_d")
        nc.sync.dma_start(out=al_d[:], in_=al_sc[:])
        nc.gpsimd.dma_start(out=alpha_sb[:], in_=al_d[:].to_broadcast((P, 1)))
        nc.vector.reciprocal(inv_alpha[:], alpha_sb[:])
        nc.scalar.mul(alpha127[:], alpha_sb[:], 1.0 / 127.0)

        # ---- phase W-B: ternarize full W^T -> fp8 wT (no device transpose):
        # scale (ACT), clamp (Pool), round straight to fp8 (Pool) -----------
        for c in range(KC):
            wc = wload.tile([P, D_OUT], F32, tag="wchunk")
            nc.scalar.dma_start(out=wc[:], in_=wt_d[ts(c, P), :])
            nc.scalar.activation(wc[:], wc[:], Copy, scale=inv_alpha[:])
            nc.gpsimd.tensor_scalar(
                wc[:], wc[:], CLAMP, -CLAMP, op0=Alu.min, op1=Alu.max
            )
            nc.gpsimd.tensor_scalar(
                wT[:, c, :], wc[:], MAGIC, MAGIC, op0=Alu.add, op1=Alu.subtract
            )

        # ---- main token loop: supertiles of st*128 tokens ---------------
        for m in range(MS):
            x_t = xin.tile([P, st, D_IN], F32, tag="x")
            nc.sync.dma_start(out=x_t[:], in_=x_v[m])

            absmax = scl.tile([P, st], F32, tag="absmax")
            m1 = scl.tile([P, st], F32, tag="m1")
            r = scl.tile([P, st], F32, tag="r")
            inv127 = scl.tile([P, st], F32, tag="inv127")
            c_vec = scl.tile([P, st], F32, tag="c_vec")

            nc.vector.tensor_reduce(
                absmax[:], x_t[:], axis=AX.X, op=Alu.max, apply_absolute_value=True
            )
            nc.gpsimd.tensor_scalar(m1[:], absmax[:], EPS, None, op0=Alu.max)
            nc.vector.reciprocal(r[:], m1[:])
            nc.scalar.mul(inv127[:], r[:], 127.0)
            nc.scalar.mul(c_vec[:], m1[:], alpha127[:])

            # q = round(x * 127/m1): ACT rounds via magic bias (f32 in-place),
            # then strips the magic -> q bf16 (also ACT; per-partition imm bias)
            for a in range(st):
                nc.scalar.activation(
                    x_t[:, a, :], x_t[:, a, :], Copy, bias=MAGIC,
                    scale=inv127[:, a : a + 1],
                )
            q_t = xq.tile([P, st, D_IN], BF16, tag="q")
            nc.scalar.activation(
                q_t.rearrange("p a b -> p (a b)"),
                x_t.rearrange("p a b -> p (a b)"), Copy, bias=-MAGIC,
            )

            # transpose to [k, tok] layout (ACT HWDGE ring)
            qT = xt.tile([P, st * KC, P], BF16, tag="qT")
            nc.scalar.dma_start_transpose(
                qT[:], q_t.rearrange("p a d -> p (a d)"))
            if USE_FP8:
                # exact split q = a8 + b8; planes [P, 2, st*KC, P] keep the
                # elementwise ops contiguous, the matmul AP picks the pair
                # with stride st*KC*P (16-aligned)
                xT8 = xq.tile([P, 2, st * KC, P], FP8, tag="xT8")
                a_pl = xT8[:, 0].rearrange("p a b -> p (a b)")
                b_pl = xT8[:, 1].rearrange("p a b -> p (a b)")
                qT_f = qT.rearrange("p a b -> p (a b)")
                nc.scalar.activation(a_pl, qT_f, Copy)
                b_eng = nc.vector if m % 2 == 0 else nc.gpsimd
                b_eng.tensor_tensor(b_pl, qT_f, a_pl, op=Alu.subtract)

            for a in range(st):
                ps = psum.tile([P, NT, NFREE], F32, tag="ps")
                for k in range(KC):
                    for n in range(NT):
                        if USE_FP8:
                            nc.tensor.matmul(
                                ps[:, n, :],
                                xT8[:, :, a * KC + k, :],
                                wT[:, k, None, ts(n, NFREE)].to_broadcast(
                                    (P, 2, NFREE)),
                                start=(k == 0), stop=(k == KC - 1),
                                perf_mode=DR,
                            )
                        else:
                            nc.tensor.matmul(
                                ps[:, n, :],
                                qT[:, a * KC + k, :],
                                wT[:, k, ts(n, NFREE)],
                                start=(k == 0), stop=(k == KC - 1),
                            )
                y_t = yout.tile([P, D_OUT], YDT, tag="y")
                ps_flat = ps.rearrange("p a b -> p (a b)")
                # fused y = ps * c_vec + bias in one DVE pass (Pool cannot
                # read PSUM)
                nc.vector.scalar_tensor_tensor(
                    y_t[:], ps_flat, c_vec[:, a : a + 1], bias_bc[:],
                    op0=Alu.mult, op1=Alu.add,
                )
                nc.gpsimd.dma_start(out=y_v[m, :, a, :], in_=y_t[:])

    nc.compile()
    return nc


_PROG_CACHE: dict[tuple, bass.Bass] = {}


def _get_prog(T: int, repeat: int = 1) -> bass.Bass:
    key = (T, repeat)
    if key not in _PROG_CACHE:
        _PROG_CACHE[key] = _build(T, repeat)
    return _PROG_CACHE[key]


def _make_in_maps(xf: np.ndarray, w: np.ndarray, b: np.ndarray, T: int):
    wt = np.ascontiguousarray(w.T)
    shard = 2 * P
    maps = []
    for c in range(N_CORES):
        m = {
            "x": np.ascontiguousarray(xf[c * T : (c + 1) * T]),
            "wt": wt,
            "b": b,
            "ws": np.ascontiguousarray(wt[c * shard : (c + 1) * shard])
                  if ALPHA_SHARD else wt,
        }
        maps.append(m)
    return maps


def kernel(x: np.ndarray, weight: np.ndarray, bias: np.ndarray) -> np.ndarray:
    orig_shape = x.shape
    xf = np.ascontiguousarray(x.reshape(-1, D_IN).astype(np.float32, copy=False))
    n_tok = xf.shape[0]
    assert n_tok % N_CORES == 0
    T = n_tok // N_CORES
    w = np.ascontiguousarray(weight.astype(np.float32, copy=False))
    b = np.ascontiguousarray(bias.astype(np.float32, copy=False))

    nc = _get_prog(T)
    in_maps = _make_in_maps(xf, w, b, T)
    res = run_bass_kernel_spmd(nc, in_maps, core_ids=list(range(N_CORES)))
    y = np.concatenate([r["y"] for r in res.results], axis=0)
    return y.reshape(orig_shape[:-1] + (D_OUT,)).astype(np.float32)


# revision 29
# speedup vs baseline: 1.0787x; 1.0787x over previous
"""BitLinear (ternary weight + per-token int8 absmax activation) on 8 trn2 cores.

y = (round(x/s) clipped) * s  @  (alpha * clip(round(W/alpha),-1,1)).T  + bias
  with s = max(absmax(x, -1), eps)/127 per token, alpha = max(mean|W|, eps).

Strategy: data-parallel over tokens (4096 tokens/core).  W^T is precomputed
HOST-side (pure layout) and passed as an extra input, so no device transpose
of W is needed; each core ternarizes the full W^T locally (one 16.8 MB load).
alpha = mean|W| is computed from each core's 1/8 shard of the chunks it loads
anyway, combined with a 4-byte AllReduce (ALPHA_SHARD=True) or fully locally
from an extra 2.1 MB pass (False).

Matmul runs in fp8e4 DoubleRow perf mode with an EXACT split of the int8
activations:  q = a + b,  a = fp8e4(q) (RNE), b = q - a (|b| <= 4, exact in
fp8e4).  The DoubleRow dual-MAC computes w*a + w*b = w*q exactly; the ternary
weight is fed to both slots via a stride-0 broadcast AP (verified exact on
hw), so wT is stored un-duplicated (32 KB/partition) and double-buffered
across repeats.  PSUM accumulates in fp32: the integer matmul is bit-exact.

Rounding uses the magic-number trick (v + 1.5*2^23 - 1.5*2^23) == RNE.
y is scaled+biased in one fused DVE op (PSUM*c + bias) and stored as bf16
(host casts back to f32; |y|<=3 so bf16 keeps rel err ~2.6e-3 << 2e-2).
"""

import numpy as np
from contextlib import ExitStack

import concourse.bass as bass
from concourse import bacc
import concourse.mybir as mybir
import concourse.tile as tile
from concourse.bass import ts
from concourse.bass_utils import run_bass_kernel_spmd

P = 128
D_IN = 2048
D_OUT = 2048
KC = D_IN // P          # 16 contraction chunks
MAGIC = 12582912.0      # 1.5 * 2**23 : fp32 RNE rounding offset
EPS = 1e-5
CLAMP = float(np.nextafter(np.float32(1.5), np.float32(0.0)))  # largest f32 < 1.5
N_CORES = 8
ST = 2                  # token tiles per supertile

USE_FP8 = False          # fp8e4 DoubleRow exact-split matmul (else bf16)
NFREE = 512            # matmul moving free size (512 | 1024 | 2048)
Y_BF16 = True           # store y as bf16 (host upcasts)
ALPHA_SHARD = True      # alpha via 1/8 shard + tiny AllReduce (else local pass)

F32 = mybir.dt.float32
BF16 = mybir.dt.bfloat16
FP8 = mybir.dt.float8e4
Copy = mybir.ActivationFunctionType.Copy
Alu = mybir.AluOpType
AX = mybir.AxisListType
DR = mybir.MatmulPerfMode.DoubleRow
GROUPS = [list(range(N_CORES))]

NT = D_OUT // NFREE     # matmuls per (token tile, k)
YDT = BF16 if Y_BF16 else F32


def _build(T: int, repeat: int = 1) -> bass.Bass:
    """Build the per-core program for T tokens (repeat>1: perf timing only)."""
    st = ST if T % (P * ST) == 0 else 1
    MS = T // (P * st)  # supertiles
    nc = bacc.Bacc(None, target_bir_lowering=False)

    N_RED = 2 if ALPHA_SHARD else KC
    x_d = nc.dram_tensor("x", [T, D_IN], F32, kind="ExternalInput")
    wt_d = nc.dram_tensor("wt", [D_IN, D_OUT], F32, kind="ExternalInput")
    ws_d = nc.dram_tensor("ws", [N_RED * P, D_OUT], F32, kind="ExternalInput")
    b_d = nc.dram_tensor("b", [D_OUT], F32, kind="ExternalInput")
    y_d = nc.dram_tensor("y", [T, D_OUT], YDT, kind="ExternalOutput")
    x_v = x_d.rearrange("(s a p) d -> s p a d", p=P, a=st)
    y_v = y_d.rearrange("(s a p) d -> s p a d", p=P, a=st)

    with tile.TileContext(nc) as tc, ExitStack() as ctx:
      const = ctx.enter_context(tc.tile_pool(name="const", bufs=1))
      alph = ctx.enter_context(tc.tile_pool(name="alph", bufs=2))
      wpool = ctx.enter_context(tc.tile_pool(name="wpool", bufs=2 if USE_FP8 else 1))
      wload = ctx.enter_context(tc.tile_pool(name="wload", bufs=3))
      xin = ctx.enter_context(tc.tile_pool(name="xin", bufs=2))
      xq = ctx.enter_context(tc.tile_pool(name="xq", bufs=2))
      xt = ctx.enter_context(tc.tile_pool(name="xt", bufs=2))
      scl = ctx.enter_context(tc.tile_pool(name="scl", bufs=4))
      yout = ctx.enter_context(tc.tile_pool(name="yout", bufs=2))
      psum = ctx.enter_context(tc.tile_pool(name="psum", bufs=2, space="PSUM"))
      dram = ctx.enter_context(tc.tile_pool(name="dram", bufs=1, space="DRAM"))

      bias_bc = const.tile([P, D_OUT], F32)
      ones = const.tile([P, 1], F32)
      nc.gpsimd.dma_start(out=bias_bc[:], in_=b_d[None, :].to_broadcast((P, D_OUT)))
      nc.scalar.activation(ones[:], bias_bc[:, 0:1], Copy, scale=0.0, bias=1.0)

      for _rep in range(repeat):
        wT = wpool.tile([P, KC, D_OUT], FP8 if USE_FP8 else BF16, tag="wT")
        partial = alph.tile([P, N_RED], F32, tag="partial")
        alpha_sb = alph.tile([P, 1], F32, tag="alpha_sb")
        inv_alpha = alph.tile([P, 1], F32, tag="inv_alpha")
        alpha127 = alph.tile([P, 1], F32, tag="alpha127")

        # ---- phase W-A: alpha from this core's shard (ALPHA_SHARD) or the
        # full W (else); finished with a tiny AllReduce when sharded --------
        for c in range(N_RED):
            sc = wload.tile([P, D_OUT], F32, tag="wchunk")
            nc.sync.dma_start(out=sc[:], in_=ws_d[ts(c, P), :])
            s1 = scl.tile([P, KC], F32, tag="s1")
            nc.vector.tensor_reduce(
                s1[:], sc.rearrange("p (a b) -> p a b", a=KC), axis=AX.X,
                op=Alu.add, apply_absolute_value=True,
            )
            nc.vector.tensor_reduce(
                partial[:, c : c + 1], s1[:], axis=AX.X, op=Alu.add
            )
        # cross-partition sum via PE dot with ones (PE adder tree), then a
        # pairwise tree over the chunk sums.
        ps0 = psum.tile([P, NT, NFREE], F32, tag="ps")
        pdot = ps0.rearrange("p a b -> p (a b)")[0:1, 0:N_RED]
        nc.tensor.matmul(pdot, ones[:], partial[:], start=True, stop=True)
        row = alph.tile([1, KC], F32, tag="row")
        nc.scalar.copy(row[0:1, 0:N_RED], pdot)
        width = N_RED // 2
        while width >= 1:
            nc.vector.tensor_tensor(
                row[0:1, 0:width], row[0:1, 0:width],
                row[0:1, width : 2 * width], op=Alu.add,
            )
            width //= 2
        if ALPHA_SHARD:
            # tiny AllReduce of the per-core shard sum
            ar_in = dram.tile([1, 1], F32, name="ar_in")
            ar_out = dram.tile([1, 1], F32, name="ar_out", addr_space="Shared")
            nc.sync.dma_start(out=ar_in[:], in_=row[0:1, 0:1])
            nc.gpsimd.collective_compute(
                "AllReduce", Alu.add, replica_groups=GROUPS,
                ins=[ar_in[:]], outs=[ar_out[:]],
            )
            tot = alph.tile([1, 1], F32, tag="tot")
            nc.sync.dma_start(out=tot[:], in_=ar_out[:])
            src = tot
        else:
            src = row
        al_sc = alph.tile([1, 1], F32, tag="al_sc")
        nc.vector.tensor_scalar(
            al_sc[:], src[0:1, 0:1], 1.0 / (D_IN * D_OUT), EPS,
            op0=Alu.mult, op1=Alu.max,
        )
        # broadcast alpha to all partitions through a DRAM bounce
        al_d = dram.tile([1, 1], F32, name="al_d")
        nc.sync.dma_start(out=al_d[:], in_=al_sc[:])
        nc.gpsimd.dma_start(out=alpha_sb[:], in_=al_d[:].to_broadcast((P, 1)))
        nc.vector.reciprocal(inv_alpha[:], alpha_sb[:])
        nc.scalar.mul(alpha127[:], alpha_sb[:], 1.0 / 127.0)

        # ---- phase W-B: ternarize full W^T -> fp8 wT (no device transpose):
        # scale (ACT), clamp (Pool), round straight to fp8 (Pool) -----------
        for c in range(KC):
            wc = wload.tile([P, D_OUT], F32, tag="wchunk")
            nc.scalar.dma_start(out=wc[:], in_=wt_d[ts(c, P), :])
            nc.scalar.activation(wc[:], wc[:], Copy, scale=inv_alpha[:])
            nc.gpsimd.tensor_scalar(
                wc[:], wc[:], CLAMP, -CLAMP, op0=Alu.min, op1=Alu.max
            )
            nc.gpsimd.tensor_scalar(
                wT[:, c, :], wc[:], MAGIC, MAGIC, op0=Alu.add, op1=Alu.subtract
            )

        # ---- main token loop: supertiles of st*128 tokens ---------------
        for m in range(MS):
            x_t = xin.tile([P, st, D_IN], F32, tag="x")
            nc.sync.dma_start(out=x_t[:], in_=x_v[m])

            absmax = scl.tile([P, st], F32, tag="absmax")
            m1 = scl.tile([P, st], F32, tag="m1")
            r = scl.tile([P, st], F32, tag="r")
            inv127 = scl.tile([P, st], F32, tag="inv127")
            c_vec = scl.tile([P, st], F32, tag="c_vec")

            nc.vector.tensor_reduce(
                absmax[:], x_t[:], axis=AX.X, op=Alu.max, apply_absolute_value=True
            )
            nc.gpsimd.tensor_scalar(m1[:], absmax[:], EPS, None, op0=Alu.max)
            nc.vector.reciprocal(r[:], m1[:])
            nc.scalar.mul(inv127[:], r[:], 127.0)
            nc.scalar.mul(c_vec[:], m1[:], alpha127[:])

            # q = round(x * 127/m1): ACT rounds via magic bias (f32 in-place),
            # then strips the magic -> q bf16 (also ACT; per-partition imm bias)
            for a in range(st):
                nc.scalar.activation(
                    x_t[:, a, :], x_t[:, a, :], Copy, bias=MAGIC,
                    scale=inv127[:, a : a + 1],
                )
            q_t = xq.tile([P, st, D_IN], BF16, tag="q")
            nc.scalar.activation(
                q_t.rearrange("p a b -> p (a b)"),
                x_t.rearrange("p a b -> p (a b)"), Copy, bias=-MAGIC,
            )

            # transpose to [k, tok] layout (ACT HWDGE ring)
            qT = xt.tile([P, st * KC, P], BF16, tag="qT")
            nc.scalar.dma_start_transpose(
                qT[:], q_t.rearrange("p a d -> p (a d)"))
            if USE_FP8:
                # exact split q = a8 + b8; planes [P, 2, st*KC, P] keep the
                # elementwise ops contiguous, the matmul AP picks the pair
                # with stride st*KC*P (16-aligned)
                xT8 = xq.tile([P, 2, st * KC, P], FP8, tag="xT8")
                a_pl = xT8[:, 0].rearrange("p a b -> p (a b)")
                b_pl = xT8[:, 1].rearrange("p a b -> p (a b)")
                qT_f = qT.rearrange("p a b -> p (a b)")
                nc.scalar.activation(a_pl, qT_f, Copy)
                b_eng = nc.vector if m % 2 == 0 else nc.gpsimd
                b_eng.tensor_tensor(b_pl, qT_f, a_pl, op=Alu.subtract)

            for a in range(st):
                ps = psum.tile([P, NT, NFREE], F32, tag="ps")
                for k in range(KC):
                    for n in range(NT):
                        if USE_FP8:
                            nc.tensor.matmul(
                                ps[:, n, :],
                                xT8[:, :, a * KC + k, :],
                                wT[:, k, None, ts(n, NFREE)].to_broadcast(
                                    (P, 2, NFREE)),
                                start=(k == 0), stop=(k == KC - 1),
                                perf_mode=DR,
                            )
                        else:
                            nc.tensor.matmul(
                                ps[:, n, :],
                                qT[:, a * KC + k, :],
                                wT[:, k, ts(n, NFREE)],
                                start=(k == 0), stop=(k == KC - 1),
                            )
                y_t = yout.tile([P, D_OUT], YDT, tag="y")
                ps_flat = ps.rearrange("p a b -> p (a b)")
                # fused y = ps * c_vec + bias in one DVE pass (Pool cannot
                # read PSUM)
                nc.vector.scalar_tensor_tensor(
                    y_t[:], ps_flat, c_vec[:, a : a + 1], bias_bc[:],
                    op0=Alu.mult, op1=Alu.add,
                )
                nc.gpsimd.dma_start(out=y_v[m, :, a, :], in_=y_t[:])

    nc.compile()
    return nc


_PROG_CACHE: dict[tuple, bass.Bass] = {}


def _get_prog(T: int, repeat: int = 1) -> bass.Bass:
    key = (T, repeat)
    if key not in _PROG_CACHE:
        _PROG_CACHE[key] = _build(T, repeat)
    return _PROG_CACHE[key]


def _make_in_maps(xf: np.ndarray, w: np.ndarray, b: np.ndarray, T: int):
    wt = np.ascontiguousarray(w.T)
    shard = 2 * P
    maps = []
    for c in range(N_CORES):
        m = {
            "x": np.ascontiguousarray(xf[c * T : (c + 1) * T]),
            "wt": wt,
            "b": b,
            "ws": np.ascontiguousarray(wt[c * shard : (c + 1) * shard])
                  if ALPHA_SHARD else wt,
        }
        maps.append(m)
    return maps


def kernel(x: np.ndarray, weight: np.ndarray, bias: np.ndarray) -> np.ndarray:
    orig_shape = x.shape
    xf = np.ascontiguousarray(x.reshape(-1, D_IN).astype(np.float32, copy=False))
    n_tok = xf.shape[0]
    assert n_tok % N_CORES == 0
    T = n_tok // N_CORES
    w = np.ascontiguousarray(weight.astype(np.float32, copy=False))
    b = np.ascontiguousarray(bias.astype(np.float32, copy=False))

    nc = _get_prog(T)
    in_maps = _make_in_maps(xf, w, b, T)
    res = run_bass_kernel_spmd(nc, in_maps, core_ids=list(range(N_CORES)))
    y = np.concatenate([r["y"] for r in res.results], axis=0)
    return y.reshape(orig_shape[:-1] + (D_OUT,)).astype(np.float32)


# revision 31
# speedup vs baseline: 2.3684x; 2.1956x over previous
"""BitLinear (ternary weight + per-token int8 absmax activation) on 8 trn2 cores.

y = (round(x/s) clipped) * s  @  (alpha * clip(round(W/alpha),-1,1)).T  + bias
  with s = max(absmax(x, -1), eps)/127 per token, alpha = max(mean|W|, eps).

Strategy: data-parallel over tokens (4096 tokens/core); weight prep sharded
over out_features (256 rows/core).  The host passes each core its W^T shard
(pure layout change, free), so there is NO device-side weight transpose:
the shard arrives as [D_IN, 256] = [128k x KC x 256o], is reduced for alpha
(PE ones-dot -> 4-byte AllReduce), ternarized in place (ACT scale, Pool
clamp+round), and AllGathered as bf16 so every core holds the full W^T.

Matmul: bf16, integer-exact (|q|<=127 and ternary w are exact in bf16, PSUM
accumulates fp32).  An fp8e4 DoubleRow path (exact q = fp8(q) + residual
split, stride-0 broadcast weight slots) is kept behind USE_FP8 but is slower
on real hw: DoubleRow disables FWL so every matmul pays an exposed 256-col
LDWEIGHTS.

Rounding uses the magic-number trick (v + 1.5*2^23 - 1.5*2^23) == RNE.
y is scaled+biased in one fused DVE op (PSUM*c + bias) and stored as bf16
(host upcasts; |y|<=3 so bf16 costs rel err ~2.6e-3 << 2e-2 budget).
"""

import numpy as np
from contextlib import ExitStack

import concourse.bass as bass
from concourse import bacc
import concourse.mybir as mybir
import concourse.tile as tile
from concourse.bass import ts
from concourse.bass_utils import run_bass_kernel_spmd

P = 128
D_IN = 2048
D_OUT = 2048
KC = D_IN // P          # 16 contraction chunks
MAGIC = 12582912.0      # 1.5 * 2**23 : fp32 RNE rounding offset
EPS = 1e-5
CLAMP = float(np.nextafter(np.float32(1.5), np.float32(0.0)))  # largest f32 < 1.5
N_CORES = 8
SH = D_OUT // N_CORES   # 256 out-features per core (weight shard)
ST = 2                  # token tiles per supertile

USE_FP8 = False         # fp8e4 DoubleRow exact-split matmul (else bf16)
NFREE = 512             # matmul moving free size (<=512: one PSUM bank)
Y_BF16 = True           # store y as bf16 (host upcasts)

F32 = mybir.dt.float32
BF16 = mybir.dt.bfloat16
FP8 = mybir.dt.float8e4
Copy = mybir.ActivationFunctionType.Copy
Alu = mybir.AluOpType
AX = mybir.AxisListType
DR = mybir.MatmulPerfMode.DoubleRow
GROUPS = [list(range(N_CORES))]

NT = D_OUT // NFREE     # matmuls per (token tile, k)
YDT = BF16 if Y_BF16 else F32
WDT = FP8 if USE_FP8 else BF16


def _build(T: int, repeat: int = 1) -> bass.Bass:
    """Build the per-core program for T tokens (repeat>1: perf timing only)."""
    st = ST if T % (P * ST) == 0 else 1
    MS = T // (P * st)  # supertiles
    nc = bacc.Bacc(None, target_bir_lowering=False)

    x_d = nc.dram_tensor("x", [T, D_IN], F32, kind="ExternalInput")
    ws_d = nc.dram_tensor("ws", [D_IN, SH], F32, kind="ExternalInput")
    b_d = nc.dram_tensor("b", [D_OUT], F32, kind="ExternalInput")
    y_d = nc.dram_tensor("y", [T, D_OUT], YDT, kind="ExternalOutput")
    x_v = x_d.rearrange("(s a p) d -> s p a d", p=P, a=st)
    y_v = y_d.rearrange("(s a p) d -> s p a d", p=P, a=st)
    ws_v = ws_d.rearrange("(c p) o -> p c o", p=P)   # [P, KC, SH]

    with tile.TileContext(nc) as tc, ExitStack() as ctx:
      const = ctx.enter_context(tc.tile_pool(name="const", bufs=1))
      alph = ctx.enter_context(tc.tile_pool(name="alph", bufs=2))
      wpool = ctx.enter_context(tc.tile_pool(name="wpool", bufs=2 if USE_FP8 else 1))
      wload = ctx.enter_context(tc.tile_pool(name="wload", bufs=1))
      wtmp = ctx.enter_context(tc.tile_pool(name="wtmp", bufs=1))
      xin = ctx.enter_context(tc.tile_pool(name="xin", bufs=2))
      xq = ctx.enter_context(tc.tile_pool(name="xq", bufs=2))
      xt = ctx.enter_context(tc.tile_pool(name="xt", bufs=2))
      scl = ctx.enter_context(tc.tile_pool(name="scl", bufs=4))
      yout = ctx.enter_context(tc.tile_pool(name="yout", bufs=2))
      psum = ctx.enter_context(tc.tile_pool(name="psum", bufs=2, space="PSUM"))
      dram = ctx.enter_context(tc.tile_pool(name="dram", bufs=1, space="DRAM"))

      bias_bc = const.tile([P, D_OUT], F32)
      ones = const.tile([P, 1], F32)
      nc.gpsimd.dma_start(out=bias_bc[:], in_=b_d[None, :].to_broadcast((P, D_OUT)))
      nc.scalar.activation(ones[:], bias_bc[:, 0:1], Copy, scale=0.0, bias=1.0)

      for _rep in range(repeat):
        wT = wpool.tile([P, KC, D_OUT], WDT, tag="wT")
        partial = alph.tile([P, 1], F32, tag="partial")
        alpha_sb = alph.tile([P, 1], F32, tag="alpha_sb")
        inv_alpha = alph.tile([P, 1], F32, tag="inv_alpha")
        alpha127 = alph.tile([P, 1], F32, tag="alpha127")

        # ---- phase W-A: alpha from this core's W^T shard + 4B AllReduce ---
        wsh = wload.tile([P, KC, SH], F32, tag="wsh")
        nc.sync.dma_start(out=wsh[:], in_=ws_v[:, :, :])
        s1 = scl.tile([P, KC], F32, tag="s1")
        nc.vector.tensor_reduce(
            s1[:], wsh[:], axis=AX.X, op=Alu.add, apply_absolute_value=True
        )
        nc.vector.tensor_reduce(partial[:], s1[:], axis=AX.X, op=Alu.add)
        # cross-partition sum via PE dot with ones (PE adder tree)
        ps0 = psum.tile([P, NT, NFREE], F32, tag="ps")
        pdot = ps0.rearrange("p a b -> p (a b)")[0:1, 0:1]
        nc.tensor.matmul(pdot, ones[:], partial[:], start=True, stop=True)
        al_pre = alph.tile([1, 1], F32, tag="al_pre")
        nc.scalar.copy(al_pre[:], pdot)
        ar_in = dram.tile([1, 1], F32, name="ar_in")
        ar_out = dram.tile([1, 1], F32, name="ar_out", addr_space="Shared")
        nc.sync.dma_start(out=ar_in[:], in_=al_pre[:])
        nc.gpsimd.collective_compute(
            "AllReduce", Alu.add, replica_groups=GROUPS,
            ins=[ar_in[:]], outs=[ar_out[:]],
        )
        tot = alph.tile([1, 1], F32, tag="tot")
        nc.sync.dma_start(out=tot[:], in_=ar_out[:])
        al_sc = alph.tile([1, 1], F32, tag="al_sc")
        nc.vector.tensor_scalar(
            al_sc[:], tot[0:1, 0:1], 1.0 / (D_IN * D_OUT), EPS,
            op0=Alu.mult, op1=Alu.max,
        )
        # broadcast alpha to all partitions through a DRAM bounce
        al_d = dram.tile([1, 1], F32, name="al_d")
        nc.sync.dma_start(out=al_d[:], in_=al_sc[:])
        nc.gpsimd.dma_start(out=alpha_sb[:], in_=al_d[:].to_broadcast((P, 1)))
        nc.vector.reciprocal(inv_alpha[:], alpha_sb[:])
        nc.scalar.mul(alpha127[:], alpha_sb[:], 1.0 / 127.0)

        # ---- phase W-B: ternarize shard in place + AllGather --------------
        # scale (ACT), clamp (Pool), round -> bf16/fp8 (Pool), gather
        wsh_f = wsh.rearrange("p a b -> p (a b)")
        nc.scalar.activation(wsh_f, wsh_f, Copy, scale=inv_alpha[:])
        nc.gpsimd.tensor_scalar(
            wsh_f, wsh_f, CLAMP, -CLAMP, op0=Alu.min, op1=Alu.max
        )
        wtn = wtmp.tile([P, KC, SH], WDT, tag="wtn")
        nc.gpsimd.tensor_scalar(
            wtn.rearrange("p a b -> p (a b)"), wsh_f, MAGIC, MAGIC,
            op0=Alu.add, op1=Alu.subtract,
        )
        contrib = dram.tile([P, KC, SH], WDT, name="contrib")
        gathered = dram.tile([N_CORES, P, KC, SH], WDT, name="gathered",
                             addr_space="Shared")
        nc.sync.dma_start(out=contrib[:], in_=wtn[:])
        nc.gpsimd.collective_compute(
            "AllGather", Alu.bypass, replica_groups=GROUPS,
            ins=[contrib[:]], outs=[gathered[:]],
        )
        for c in range(N_CORES):
            nc.scalar.dma_start(out=wT[:, :, ts(c, SH)], in_=gathered[c])

        # ---- main token loop: supertiles of st*128 tokens ---------------
        for m in range(MS):
            x_t = xin.tile([P, st, D_IN], F32, tag="x")
            nc.sync.dma_start(out=x_t[:], in_=x_v[m])

            absmax = scl.tile([P, st], F32, tag="absmax")
            m1 = scl.tile([P, st], F32, tag="m1")
            r = scl.tile([P, st], F32, tag="r")
            inv127 = scl.tile([P, st], F32, tag="inv127")
            c_vec = scl.tile([P, st], F32, tag="c_vec")

            nc.vector.tensor_reduce(
                absmax[:], x_t[:], axis=AX.X, op=Alu.max, apply_absolute_value=True
            )
            nc.gpsimd.tensor_scalar(m1[:], absmax[:], EPS, None, op0=Alu.max)
            nc.vector.reciprocal(r[:], m1[:])
            nc.scalar.mul(inv127[:], r[:], 127.0)
            nc.scalar.mul(c_vec[:], m1[:], alpha127[:])

            # q = round(x * 127/m1): ACT rounds via magic bias (f32 in-place),
            # then strips the magic -> q bf16 (also ACT)
            for a in range(st):
                nc.scalar.activation(
                    x_t[:, a, :], x_t[:, a, :], Copy, bias=MAGIC,
                    scale=inv127[:, a : a + 1],
                )
            q_t = xq.tile([P, st, D_IN], BF16, tag="q")
            nc.scalar.activation(
                q_t.rearrange("p a b -> p (a b)"),
                x_t.rearrange("p a b -> p (a b)"), Copy, bias=-MAGIC,
            )

            # transpose to [k, tok] layout (ACT HWDGE ring)
            qT = xt.tile([P, st * KC, P], BF16, tag="qT")
            nc.scalar.dma_start_transpose(
                qT[:], q_t.rearrange("p a d -> p (a d)"))
            if USE_FP8:
                # exact split q = a8 + b8; planes [P, 2, st*KC, P]
                xT8 = xq.tile([P, 2, st * KC, P], FP8, tag="xT8")
                a_pl = xT8[:, 0].rearrange("p a b -> p (a b)")
                b_pl = xT8[:, 1].rearrange("p a b -> p (a b)")
                qT_f = qT.rearrange("p a b -> p (a b)")
                nc.scalar.activation(a_pl, qT_f, Copy)
                b_eng = nc.vector if m % 2 == 0 else nc.gpsimd
                b_eng.tensor_tensor(b_pl, qT_f, a_pl, op=Alu.subtract)

            for a in range(st):
                ps = psum.tile([P, NT, NFREE], F32, tag="ps")
                # first supertile: n-outer so the n=0 matmuls only need the
                # first gather slices -- PE starts before full wT staging.
                loop = (
                    [(k, n) for n in range(NT) for k in range(KC)]
                    if m == 0 else
                    [(k, n) for k in range(KC) for n in range(NT)]
                )
                for k, n in loop:
                    if USE_FP8:
                        nc.tensor.matmul(
                            ps[:, n, :],
                            xT8[:, :, a * KC + k, :],
                            wT[:, k, None, ts(n, NFREE)].to_broadcast(
                                (P, 2, NFREE)),
                            start=(k == 0), stop=(k == KC - 1),
                            perf_mode=DR,
                        )
                    else:
                        nc.tensor.matmul(
                            ps[:, n, :],
                            qT[:, a * KC + k, :],
                            wT[:, k, ts(n, NFREE)],
                            start=(k == 0), stop=(k == KC - 1),
                        )
                y_t = yout.tile([P, D_OUT], YDT, tag="y")
                ps_flat = ps.rearrange("p a b -> p (a b)")
                # fused y = ps * c_vec + bias in one DVE pass
                nc.vector.scalar_tensor_tensor(
                    y_t[:], ps_flat, c_vec[:, a : a + 1], bias_bc[:],
                    op0=Alu.mult, op1=Alu.add,
                )
                nc.sync.dma_start(out=y_v[m, :, a, :], in_=y_t[:])

    nc.compile()
    return nc


_PROG_CACHE: dict[tuple, bass.Bass] = {}


def _get_prog(T: int, repeat: int = 1) -> bass.Bass:
    key = (T, repeat)
    if key not in _PROG_CACHE:
        _PROG_CACHE[key] = _build(T, repeat)
    return _PROG_CACHE[key]


def _make_in_maps(xf: np.ndarray, w: np.ndarray, b: np.ndarray, T: int):
    wt = np.ascontiguousarray(w.T)
    return [
        {
            "x": np.ascontiguousarray(xf[c * T : (c + 1) * T]),
            "ws": np.ascontiguousarray(wt[:, c * SH : (c + 1) * SH]),
            "b": b,
        }
        for c in range(N_CORES)
    ]


def kernel(x: np.ndarray, weight: np.ndarray, bias: np.ndarray) -> np.ndarray:
    orig_shape = x.shape
    xf = np.ascontiguousarray(x.reshape(-1, D_IN).astype(np.float32, copy=False))
    n_tok = xf.shape[0]
    assert n_tok % N_CORES == 0
    T = n_tok // N_CORES
    w = np.ascontiguousarray(weight.astype(np.float32, copy=False))
    b = np.ascontiguousarray(bias.astype(np.float32, copy=False))

    nc = _get_prog(T)
    in_maps = _make_in_maps(xf, w, b, T)
    res = run_bass_kernel_spmd(nc, in_maps, core_ids=list(range(N_CORES)))
    y = np.concatenate([r["y"] for r in res.results], axis=0)
    return y.reshape(orig_shape[:-1] + (D_OUT,)).astype(np.float32)
